# revision 14
# baseline (speedup 1.0000x reference)
"""Causal attention (B=4, S=4096, D=64, fp32) on 8 TRN2 NeuronCores.

Sharding: 8 cores = 4 batches x 2 query-parity shards. Core (b, p) handles
query rows  q_global = 2*i + p  of batch b (i = 0..2047). This interleaved
split makes the causal extent pattern identical on every core (SPMD-uniform):
local query block qb (128 rows) attends exactly key blocks 0..2*qb+1, so key
blocks are processed in PAIRS j = (2j, 2j+1), both with extent [128j, 2048).

Per-core kernel (matmul operands bf16, accumulation fp32), loop over j=0..15:
  S^T[k, q] for kb=2j / 2j+1 as two row-group-packed matmuls (K=64 each,
    PE rows 0-63 / 64-127 concurrently), outputs side by side in one PSUM
    tile [128, A|B]                                              (PE)
  E = exp(S^T * 0.125 [+ pad bias])   one ACT op per chunk covers both kb
  E[:, diag] *= causal mask           (DVE, strided AP hits both kb)
  PV[q, 0:65] += E_even^T @ [V|1] + E_odd^T @ [V|1]  (PE; ones col = Z)
  out[q, :] = PV[q, :64] * (1 / PV[q, 64])

Emission is software-pipelined: S^T(j) is issued before PV(j-1) so the PE
queue always has independent work while ACT(j-1) finishes.

No max-subtraction: scaled scores are ~N(0,1), exp is safe in fp32. The
softmax denominator comes from the ones column, so numerator and denominator
use identical bf16 weights.
"""

import numpy as np
import ml_dtypes

import concourse.bass as bass
import concourse.bacc as bacc
import concourse.mybir as mybir
import concourse.tile as tile
from concourse.bass_utils import run_bass_kernel_spmd

BF16 = mybir.dt.bfloat16
F32 = mybir.dt.float32
NP_BF16 = ml_dtypes.bfloat16

B, S, D = 4, 4096, 64
P = 128
SL = S // 2          # local query count per core
NKB = S // P         # 32 key blocks
NPAIR = NKB // 2     # 16 key-block pairs
NQB = SL // P        # 16 local query blocks
SCALE = 1.0 / np.sqrt(D)
PAD_BIAS = -50.0     # additive pre-exp bias for padded-out keys
N_CORES = 8

_cache: dict = {}


def _chunks(extent):
    """Split [0, extent) into a leading remainder chunk (if any) plus full
    512-col chunks, so every chunk boundary is 512-aligned from the top."""
    rem = extent % 512
    out = []
    c = 0
    if rem:
        out.append((0, rem))
        c = rem
    while c < extent:
        out.append((c, 512))
        c += 512
    return out


def _build_program(with_padding: bool):
    nc = bacc.Bacc("TRN2", debug=False)

    # Host pre-layouts (see kernel()):
    #  qt2 [128, 2048]: rows 0-63 = Q^T, rows 64-127 = the same Q^T again
    #  kt2 [128, 2048]: rows 0-63 = K^T of even key blocks, 64-127 = odd
    #  v1  [128, 32*65]: row p = concat_kb [V[kb*128+p, :], 1.0]
    qt2 = nc.dram_tensor("qt2", [P, SL], BF16, kind="ExternalInput")
    kt2 = nc.dram_tensor("kt2", [P, SL], BF16, kind="ExternalInput")
    v1 = nc.dram_tensor("v1", [P, NKB * (D + 1)], BF16, kind="ExternalInput")
    dmask = nc.dram_tensor("dmask", [P, 2 * P], BF16, kind="ExternalInput")
    if with_padding:
        biasm = nc.dram_tensor("biasm", [P, NKB], F32, kind="ExternalInput")
    out = nc.dram_tensor("out", [SL, D], F32, kind="ExternalOutput")

    with tile.TileContext(nc) as tc:
        with (
            tc.tile_pool(name="const", bufs=1) as constp,
            tc.tile_pool(name="spool", bufs=2, space="PSUM") as spool,
            tc.tile_pool(name="opsum", bufs=1, space="PSUM") as opsum,
            tc.tile_pool(name="epool", bufs=8) as epool,
            tc.tile_pool(name="npool", bufs=4) as npool,
        ):
            # Chunked input loads on separate DGE queues so the first S^T
            # matmul is gated only on the first kt/qt chunks, not the full
            # tensors. kt chunks on SP, qt chunks on ACT, mask+V on GPSIMD.
            qt_t = constp.tile([P, SL], BF16, tag="qt")
            kt_t = constp.tile([P, SL], BF16, tag="kt")
            for c in range(0, SL, 512):
                nc.sync.dma_start(kt_t[:, c:c + 512], kt2[:, c:c + 512])
                nc.scalar.dma_start(qt_t[:, c:c + 512], qt2[:, c:c + 512])
            dm_t = constp.tile([P, 2 * P], BF16, tag="dmask")
            nc.gpsimd.dma_start(dm_t[:], dmask[:])
            v1_t = constp.tile([P, NKB, D + 1], BF16, tag="v1")
            nc.gpsimd.dma_start(
                v1_t[:], v1[:].rearrange("p (kb c) -> p kb c", kb=NKB)
            )
            if with_padding:
                bm_t = constp.tile([P, NKB], F32, tag="biasm")
                nc.scalar.dma_start(bm_t[:], biasm[:])

            # 4 PSUM banks holding the 16 query-block accumulators
            # [128, 65] side by side. Uneven packing {7, 6, 2, 1} makes the
            # accumulation groups close at pairs 6 / 12 / 14 / 15, so all
            # but one normalization happens mid-kernel, not in the tail.
            BANK_OF = [0] * 7 + [1] * 6 + [2] * 2 + [3]
            BANK_START = [0, 7, 13, 15]
            BANK_END = [6, 12, 14, 15]
            ob = [
                opsum.tile([P, 512], F32, tag=f"ob{j}", name=f"ob{j}")
                for j in range(4)
            ]

            def emit_pv(j):
                # PV matmuls for key pair j: qb = j..15, even then odd kb.
                q0 = j * P
                extent = SL - q0
                ch = _chunks(extent)
                for parity in range(2):
                    kb = 2 * j + parity
                    for qb in range(j, NQB):
                        off = (qb - j) * P
                        # locate chunk containing off
                        for ci, (c0, clen) in enumerate(ch):
                            if c0 <= off < c0 + clen:
                                break
                        e = e_tiles[(j % 2, ci)]
                        col = parity * 512 + (off - c0)
                        bank = BANK_OF[qb]
                        slot = qb - BANK_START[bank]
                        nc.tensor.matmul(
                            ob[bank][:, slot * 65: slot * 65 + 65],
                            e[:, col: col + P],
                            v1_t[:, kb, :],
                            start=(j == 0 and parity == 0 and slot == 0),
                            stop=(j == BANK_END[bank] and parity == 1
                                  and qb == BANK_END[bank]),
                        )

            def emit_norm(qb):
                bank = BANK_OF[qb]
                slot = qb - BANK_START[bank]
                r = npool.tile([P, 1], F32, tag="r", name="r")
                nc.vector.reciprocal(
                    r[:], ob[bank][:, slot * 65 + 64: slot * 65 + 65]
                )
                o = npool.tile([P, D], F32, tag="o", name="o")
                nc.vector.tensor_scalar_mul(
                    o[:], ob[bank][:, slot * 65: slot * 65 + 64], r[:]
                )
                nc.sync.dma_start(out[qb * P:(qb + 1) * P, :], o[:])

            # main software-pipelined loop over key-block pairs
            e_tiles = {}
            pending_norms = []
            for j in range(NPAIR):
                q0 = j * P
                extent = SL - q0
                ch = _chunks(extent)
                # S^T for pair j: two row-group-packed matmuls per chunk
                # (even kb on PE rows 0-63, odd kb on rows 64-127)
                ps_tiles = []
                for ci, (c0, clen) in enumerate(ch):
                    ps = spool.tile([P, 1024], F32, tag="ps", name="ps")
                    ps_tiles.append(ps)
                    for parity in range(2):
                        lo = parity * 64
                        nc.tensor.matmul(
                            ps[:, parity * 512: parity * 512 + clen],
                            kt_t[lo:lo + 64, j * P:(j + 1) * P],
                            qt_t[lo:lo + 64, q0 + c0: q0 + c0 + clen],
                            start=True, stop=True,
                        )
                # PV for the previous pair (keeps PE busy during ACT(j)).
                if j > 0:
                    emit_pv(j - 1)
                    for bank in range(4):
                        if BANK_END[bank] == j - 1:
                            pending_norms.extend(
                                range(BANK_START[bank], BANK_END[bank] + 1))
                # Normalizations are spread out (max 2 per pair) so the DVE
                # queue never delays the diag-mask op by a long norm burst.
                for _ in range(2):
                    if pending_norms:
                        emit_norm(pending_norms.pop(0))
                # exp for pair j
                for ci, (c0, clen) in enumerate(ch):
                    ps = ps_tiles[ci]
                    e = epool.tile([P, 1024], BF16, tag="e", name="e")
                    e_tiles[(j % 2, ci)] = e
                    if with_padding:
                        # separate exp per kb: bias differs per parity
                        for parity in range(2):
                            nc.scalar.activation(
                                e[:, parity * 512: parity * 512 + clen],
                                ps[:, parity * 512: parity * 512 + clen],
                                mybir.ActivationFunctionType.Exp,
                                bias=bm_t[:, 2 * j + parity: 2 * j + parity + 1],
                                scale=float(SCALE),
                            )
                    else:
                        if clen == 512:
                            src, dst = ps[:, :1024], e[:, :1024]
                        else:
                            # strided AP: [0:clen] and [512:512+clen]
                            src = ps[:].rearrange(
                                "p (two f) -> p two f", two=2)[:, :, :clen]
                            dst = e[:].rearrange(
                                "p (two f) -> p two f", two=2)[:, :, :clen]
                        nc.scalar.activation(
                            dst, src,
                            mybir.ActivationFunctionType.Exp,
                            bias=0.0,
                            scale=float(SCALE),
                        )
                # causal mask on the two diagonal blocks (first 128 q cols):
                # chunk 0 holds them at cols [0:128] (even) / [512:640] (odd)
                e0 = e_tiles[(j % 2, 0)]
                ea = e0[:].rearrange("p (two f) -> p two f", two=2)[:, :, :P]
                ma = dm_t[:].rearrange("p (two f) -> p two f", two=2)
                nc.vector.tensor_mul(ea, ea, ma)

            emit_pv(NPAIR - 1)
            for qb in pending_norms:
                emit_norm(qb)
            emit_norm(NPAIR - 1)  # bank 3 = qb 15 only

    nc.compile()
    return nc


def _get_program(with_padding: bool):
    key = ("prog", with_padding)
    if key not in _cache:
        _cache[key] = _build_program(with_padding)
    return _cache[key]


def _diag_masks():
    # dmask[:, 0:128]  : key block 2*qb   -> visible iff u <= 2r+p
    # dmask[:, 128:256]: key block 2*qb+1 -> visible iff u+128 <= 2r+p
    u = np.arange(P)[:, None]
    r = np.arange(P)[None, :]
    out = []
    for p in range(2):
        m0 = (u <= 2 * r + p)
        m1 = (u + P <= 2 * r + p)
        out.append(np.concatenate([m0, m1], axis=1).astype(NP_BF16))
    return out


def kernel(query, key, value, attention_mask, _run_opts=None):
    query = np.asarray(query, dtype=np.float32)
    key = np.asarray(key, dtype=np.float32)
    value = np.asarray(value, dtype=np.float32)
    attention_mask = np.asarray(attention_mask)

    with_padding = not bool((attention_mask != 0).all())
    nc = _get_program(with_padding)
    masks = _diag_masks()

    in_maps = []
    for b in range(B):
        kt = key[b].T  # [64, 4096]
        # kt2: top half = even key blocks, bottom half = odd key blocks
        ktb = kt.reshape(D, NPAIR, 2, P)
        kt2_b = np.ascontiguousarray(
            np.concatenate([ktb[:, :, 0, :], ktb[:, :, 1, :]], axis=0)
            .reshape(2 * D, SL).astype(NP_BF16))
        # v1: [128, 32*65], row p = concat over kb of [V[kb*128+p, :], 1]
        v1f = np.concatenate(
            [value[b].reshape(NKB, P, D),
             np.ones((NKB, P, 1), np.float32)], axis=2)      # [32, 128, 65]
        v1_b = np.ascontiguousarray(
            v1f.transpose(1, 0, 2).reshape(P, NKB * (D + 1)).astype(NP_BF16))
        if with_padding:
            bias_b = np.where(
                attention_mask[b] != 0, 0.0, PAD_BIAS).astype(np.float32)
            biasm_b = np.ascontiguousarray(bias_b.reshape(NKB, P).T)
        for p in range(2):
            qt = query[b, p::2].T.astype(NP_BF16)            # [64, 2048]
            qt2_b = np.ascontiguousarray(np.concatenate([qt, qt], axis=0))
            m = {
                "qt2": qt2_b,
                "kt2": kt2_b,
                "v1": v1_b,
                "dmask": masks[p],
            }
            if with_padding:
                m["biasm"] = biasm_b
            in_maps.append(m)

    run_opts = _run_opts or {}
    res = run_bass_kernel_spmd(nc, in_maps, core_ids=list(range(N_CORES)),
                               **run_opts)
    if run_opts:
        _cache["last_results"] = res

    out = np.empty((B, S, D), np.float32)
    for i in range(N_CORES):
        b, p = divmod(i, 2)
        out[b, p::2] = res.results[i]["out"]
    return out


# revision 16
# speedup vs baseline: 1.0168x; 1.0168x over previous
"""Causal attention (B=4, S=4096, D=64, fp32) on 8 TRN2 NeuronCores.

Sharding: 8 cores = 4 batches x 2 query-parity shards. Core (b, p) handles
query rows  q_global = 2*i + p  of batch b (i = 0..2047). This interleaved
split makes the causal extent pattern identical on every core (SPMD-uniform):
local query block qb (128 rows) attends exactly key blocks 0..2*qb+1, so key
blocks are processed in PAIRS j = (2j, 2j+1), both with extent [128j, 2048).

Per-core kernel (matmul operands bf16, accumulation fp32), loop over j=0..15:
  S^T[k, q] for kb=2j / 2j+1 as two row-group-packed matmuls (K=64 each,
    PE rows 0-63 / 64-127 concurrently), outputs side by side in one PSUM
    tile [128, A|B]                                              (PE)
  E = exp(S^T * 0.125 [+ pad bias])   one ACT op per chunk covers both kb
  E[:, diag] *= causal mask           (DVE, strided AP hits both kb)
  PV[q, 0:65] += E_even^T @ [V|1] + E_odd^T @ [V|1]  (PE; ones col = Z)
  out[q, :] = PV[q, :64] * (1 / PV[q, 64])

Emission is software-pipelined: S^T(j) is issued before PV(j-1) so the PE
queue always has independent work while ACT(j-1) finishes.

No max-subtraction: scaled scores are ~N(0,1), exp is safe in fp32. The
softmax denominator comes from the ones column, so numerator and denominator
use identical bf16 weights.
"""

import numpy as np
import ml_dtypes

import concourse.bass as bass
import concourse.bacc as bacc
import concourse.mybir as mybir
import concourse.tile as tile
from concourse.bass_utils import run_bass_kernel_spmd

BF16 = mybir.dt.bfloat16
F32 = mybir.dt.float32
NP_BF16 = ml_dtypes.bfloat16

B, S, D = 4, 4096, 64
P = 128
SL = S // 2          # local query count per core
NKB = S // P         # 32 key blocks
NPAIR = NKB // 2     # 16 key-block pairs
NQB = SL // P        # 16 local query blocks
SCALE = 1.0 / np.sqrt(D)
PAD_BIAS = -50.0     # additive pre-exp bias for padded-out keys
N_CORES = 8

_cache: dict = {}


def _chunks(extent):
    """Split [0, extent) into a leading remainder chunk (if any) plus full
    512-col chunks, so every chunk boundary is 512-aligned from the top."""
    rem = extent % 512
    out = []
    c = 0
    if rem:
        out.append((0, rem))
        c = rem
    while c < extent:
        out.append((c, 512))
        c += 512
    return out


def _build_program(with_padding: bool):
    nc = bacc.Bacc("TRN2", debug=False)

    # Host pre-layouts (see kernel()):
    #  qt2 [128, 2048]: rows 0-63 = Q^T, rows 64-127 = the same Q^T again
    #  kt2 [128, 2048]: rows 0-63 = K^T of even key blocks, 64-127 = odd
    #  v1  [128, 32*65]: row p = concat_kb [V[kb*128+p, :], 1.0]
    qt2 = nc.dram_tensor("qt2", [P, SL], BF16, kind="ExternalInput")
    kt2 = nc.dram_tensor("kt2", [P, SL], BF16, kind="ExternalInput")
    v1 = nc.dram_tensor("v1", [P, NKB * (D + 1)], BF16, kind="ExternalInput")
    dmask = nc.dram_tensor("dmask", [P, 2 * P], BF16, kind="ExternalInput")
    if with_padding:
        biasm = nc.dram_tensor("biasm", [P, NKB], F32, kind="ExternalInput")
    out = nc.dram_tensor("out", [SL, D], F32, kind="ExternalOutput")

    with tile.TileContext(nc) as tc:
        with (
            tc.tile_pool(name="const", bufs=1) as constp,
            tc.tile_pool(name="spool", bufs=2, space="PSUM") as spool,
            tc.tile_pool(name="opsum", bufs=1, space="PSUM") as opsum,
            tc.tile_pool(name="epool", bufs=8) as epool,
            tc.tile_pool(name="npool", bufs=4) as npool,
        ):
            # Chunked input loads on separate DGE queues so the first S^T
            # matmul is gated only on the first kt/qt chunks, not the full
            # tensors. kt chunks on SP, qt chunks on ACT, mask+V on GPSIMD.
            qt_t = constp.tile([P, SL], BF16, tag="qt")
            kt_t = constp.tile([P, SL], BF16, tag="kt")
            for c in range(0, SL, 512):
                nc.sync.dma_start(kt_t[:, c:c + 512], kt2[:, c:c + 512])
                nc.scalar.dma_start(qt_t[:, c:c + 512], qt2[:, c:c + 512])
            dm_t = constp.tile([P, 2 * P], BF16, tag="dmask")
            nc.gpsimd.dma_start(dm_t[:], dmask[:])
            v1_t = constp.tile([P, NKB, D + 1], BF16, tag="v1")
            nc.gpsimd.dma_start(
                v1_t[:], v1[:].rearrange("p (kb c) -> p kb c", kb=NKB)
            )
            if with_padding:
                bm_t = constp.tile([P, NKB], F32, tag="biasm")
                nc.scalar.dma_start(bm_t[:], biasm[:])

            # 4 PSUM banks holding the 16 query-block accumulators
            # [128, 65] side by side. Uneven packing {7, 6, 2, 1} makes the
            # accumulation groups close at pairs 6 / 12 / 14 / 15, so all
            # but one normalization happens mid-kernel, not in the tail.
            BANK_OF = [0] * 5 + [1] * 5 + [2] * 5 + [3]
            BANK_START = [0, 5, 10, 15]
            BANK_END = [4, 9, 14, 15]
            ob = [
                opsum.tile([P, 512], F32, tag=f"ob{j}", name=f"ob{j}")
                for j in range(4)
            ]

            def emit_pv(j):
                # PV matmuls for key pair j: qb = j..15, even then odd kb.
                q0 = j * P
                extent = SL - q0
                ch = _chunks(extent)
                for parity in range(2):
                    kb = 2 * j + parity
                    for qb in range(j, NQB):
                        off = (qb - j) * P
                        # locate chunk containing off
                        for ci, (c0, clen) in enumerate(ch):
                            if c0 <= off < c0 + clen:
                                break
                        e = e_tiles[(j % 2, ci)]
                        col = parity * 512 + (off - c0)
                        bank = BANK_OF[qb]
                        slot = qb - BANK_START[bank]
                        nc.tensor.matmul(
                            ob[bank][:, slot * 65: slot * 65 + 65],
                            e[:, col: col + P],
                            v1_t[:, kb, :],
                            start=(j == 0 and parity == 0 and slot == 0),
                            stop=(j == BANK_END[bank] and parity == 1
                                  and qb == BANK_END[bank]),
                        )

            def emit_norm(qb):
                bank = BANK_OF[qb]
                slot = qb - BANK_START[bank]
                r = npool.tile([P, 1], F32, tag="r", name="r")
                nc.vector.reciprocal(
                    r[:], ob[bank][:, slot * 65 + 64: slot * 65 + 65]
                )
                o = npool.tile([P, D], F32, tag="o", name="o")
                nc.vector.tensor_scalar_mul(
                    o[:], ob[bank][:, slot * 65: slot * 65 + 64], r[:]
                )
                nc.sync.dma_start(out[qb * P:(qb + 1) * P, :], o[:])

            # main software-pipelined loop over key-block pairs
            e_tiles = {}
            pending_norms = []
            for j in range(NPAIR):
                q0 = j * P
                extent = SL - q0
                ch = _chunks(extent)
                # S^T for pair j: two row-group-packed matmuls per chunk
                # (even kb on PE rows 0-63, odd kb on rows 64-127)
                ps_tiles = []
                for ci, (c0, clen) in enumerate(ch):
                    ps = spool.tile([P, 1024], F32, tag="ps", name="ps")
                    ps_tiles.append(ps)
                    for parity in range(2):
                        lo = parity * 64
                        nc.tensor.matmul(
                            ps[:, parity * 512: parity * 512 + clen],
                            kt_t[lo:lo + 64, j * P:(j + 1) * P],
                            qt_t[lo:lo + 64, q0 + c0: q0 + c0 + clen],
                            start=True, stop=True,
                        )
                # PV for the previous pair (keeps PE busy during ACT(j)).
                if j > 0:
                    emit_pv(j - 1)
                    for bank in range(4):
                        if BANK_END[bank] == j - 1:
                            pending_norms.extend(
                                range(BANK_START[bank], BANK_END[bank] + 1))
                # Normalizations are spread out (max 2 per pair) so the DVE
                # queue never delays the diag-mask op by a long norm burst.
                for _ in range(2):
                    if pending_norms:
                        emit_norm(pending_norms.pop(0))
                # exp for pair j
                for ci, (c0, clen) in enumerate(ch):
                    ps = ps_tiles[ci]
                    e = epool.tile([P, 1024], BF16, tag="e", name="e")
                    e_tiles[(j % 2, ci)] = e
                    if with_padding:
                        # separate exp per kb: bias differs per parity
                        for parity in range(2):
                            nc.scalar.activation(
                                e[:, parity * 512: parity * 512 + clen],
                                ps[:, parity * 512: parity * 512 + clen],
                                mybir.ActivationFunctionType.Exp,
                                bias=bm_t[:, 2 * j + parity: 2 * j + parity + 1],
                                scale=float(SCALE),
                            )
                    else:
                        if clen == 512:
                            src, dst = ps[:, :1024], e[:, :1024]
                        else:
                            # strided AP: [0:clen] and [512:512+clen]
                            src = ps[:].rearrange(
                                "p (two f) -> p two f", two=2)[:, :, :clen]
                            dst = e[:].rearrange(
                                "p (two f) -> p two f", two=2)[:, :, :clen]
                        nc.scalar.activation(
                            dst, src,
                            mybir.ActivationFunctionType.Exp,
                            bias=0.0,
                            scale=float(SCALE),
                        )
                # causal mask on the two diagonal blocks (first 128 q cols):
                # chunk 0 holds them at cols [0:128] (even) / [512:640] (odd)
                e0 = e_tiles[(j % 2, 0)]
                ea = e0[:].rearrange("p (two f) -> p two f", two=2)[:, :, :P]
                ma = dm_t[:].rearrange("p (two f) -> p two f", two=2)
                nc.vector.tensor_mul(ea, ea, ma)

            emit_pv(NPAIR - 1)
            for bank in range(4):
                if BANK_END[bank] == NPAIR - 1:
                    pending_norms.extend(
                        range(BANK_START[bank], BANK_END[bank] + 1))
            for qb in pending_norms:
                emit_norm(qb)

    nc.compile()
    return nc


def _get_program(with_padding: bool):
    key = ("prog", with_padding)
    if key not in _cache:
        _cache[key] = _build_program(with_padding)
    return _cache[key]


def _diag_masks():
    # dmask[:, 0:128]  : key block 2*qb   -> visible iff u <= 2r+p
    # dmask[:, 128:256]: key block 2*qb+1 -> visible iff u+128 <= 2r+p
    u = np.arange(P)[:, None]
    r = np.arange(P)[None, :]
    out = []
    for p in range(2):
        m0 = (u <= 2 * r + p)
        m1 = (u + P <= 2 * r + p)
        out.append(np.concatenate([m0, m1], axis=1).astype(NP_BF16))
    return out


def kernel(query, key, value, attention_mask, _run_opts=None):
    query = np.asarray(query, dtype=np.float32)
    key = np.asarray(key, dtype=np.float32)
    value = np.asarray(value, dtype=np.float32)
    attention_mask = np.asarray(attention_mask)

    with_padding = not bool((attention_mask != 0).all())
    nc = _get_program(with_padding)
    masks = _diag_masks()

    in_maps = []
    for b in range(B):
        kt = key[b].T  # [64, 4096]
        # kt2: top half = even key blocks, bottom half = odd key blocks
        ktb = kt.reshape(D, NPAIR, 2, P)
        kt2_b = np.ascontiguousarray(
            np.concatenate([ktb[:, :, 0, :], ktb[:, :, 1, :]], axis=0)
            .reshape(2 * D, SL).astype(NP_BF16))
        # v1: [128, 32*65], row p = concat over kb of [V[kb*128+p, :], 1]
        v1f = np.concatenate(
            [value[b].reshape(NKB, P, D),
             np.ones((NKB, P, 1), np.float32)], axis=2)      # [32, 128, 65]
        v1_b = np.ascontiguousarray(
            v1f.transpose(1, 0, 2).reshape(P, NKB * (D + 1)).astype(NP_BF16))
        if with_padding:
            bias_b = np.where(
                attention_mask[b] != 0, 0.0, PAD_BIAS).astype(np.float32)
            biasm_b = np.ascontiguousarray(bias_b.reshape(NKB, P).T)
        for p in range(2):
            qt = query[b, p::2].T.astype(NP_BF16)            # [64, 2048]
            qt2_b = np.ascontiguousarray(np.concatenate([qt, qt], axis=0))
            m = {
                "qt2": qt2_b,
                "kt2": kt2_b,
                "v1": v1_b,
                "dmask": masks[p],
            }
            if with_padding:
                m["biasm"] = biasm_b
            in_maps.append(m)

    run_opts = _run_opts or {}
    res = run_bass_kernel_spmd(nc, in_maps, core_ids=list(range(N_CORES)),
                               **run_opts)
    if run_opts:
        _cache["last_results"] = res

    out = np.empty((B, S, D), np.float32)
    for i in range(N_CORES):
        b, p = divmod(i, 2)
        out[b, p::2] = res.results[i]["out"]
    return out


# revision 17
# speedup vs baseline: 1.0648x; 1.0471x over previous
"""Causal attention (B=4, S=4096, D=64, fp32) on 8 TRN2 NeuronCores.

Sharding: 8 cores = 4 batches x 2 query-parity shards. Core (b, p) handles
query rows  q_global = 2*i + p  of batch b (i = 0..2047). This interleaved
split makes the causal extent pattern identical on every core (SPMD-uniform):
local query block qb (128 rows) attends exactly key blocks 0..2*qb+1, so key
blocks are processed in PAIRS j = (2j, 2j+1), both with extent [128j, 2048).

Per-core kernel (matmul operands bf16, accumulation fp32), loop over j=0..15:
  S^T[k, q] for kb=2j / 2j+1 as two row-group-packed matmuls (K=64 each,
    PE rows 0-63 / 64-127 concurrently), outputs side by side in one PSUM
    tile [128, A|B]                                              (PE)
  E = exp(S^T * 0.125 [+ pad bias])   one ACT op per chunk covers both kb
  E[:, diag] *= causal mask           (DVE, strided AP hits both kb)
  PV[q, 0:65] += E_even^T @ [V|1] + E_odd^T @ [V|1]  (PE; ones col = Z)
  out[q, :] = PV[q, :64] * (1 / PV[q, 64])

Emission is software-pipelined: S^T(j) is issued before PV(j-1) so the PE
queue always has independent work while ACT(j-1) finishes.

No max-subtraction: scaled scores are ~N(0,1), exp is safe in fp32. The
softmax denominator comes from the ones column, so numerator and denominator
use identical bf16 weights.
"""

import numpy as np
import ml_dtypes

import concourse.bass as bass
import concourse.bacc as bacc
import concourse.mybir as mybir
import concourse.tile as tile
from concourse.bass_utils import run_bass_kernel_spmd

BF16 = mybir.dt.bfloat16
F32 = mybir.dt.float32
NP_BF16 = ml_dtypes.bfloat16

B, S, D = 4, 4096, 64
P = 128
SL = S // 2          # local query count per core
NKB = S // P         # 32 key blocks
NPAIR = NKB // 2     # 16 key-block pairs
NQB = SL // P        # 16 local query blocks
SCALE = 1.0 / np.sqrt(D)
PAD_BIAS = -50.0     # additive pre-exp bias for padded-out keys
N_CORES = 8

_cache: dict = {}


def _chunks(extent):
    """Split [0, extent) into a leading remainder chunk (if any) plus full
    512-col chunks, so every chunk boundary is 512-aligned from the top."""
    rem = extent % 512
    out = []
    c = 0
    if rem:
        out.append((0, rem))
        c = rem
    while c < extent:
        out.append((c, 512))
        c += 512
    return out


def _build_program(with_padding: bool):
    nc = bacc.Bacc("TRN2", debug=False)

    # Host pre-layouts (see kernel()):
    #  qt2 [128, 2048]: rows 0-63 = Q^T, rows 64-127 = the same Q^T again
    #  kt2 [128, 2048]: rows 0-63 = K^T of even key blocks, 64-127 = odd
    #  v1  [128, 32*65]: row p = concat_kb [V[kb*128+p, :], 1.0]
    qt2 = nc.dram_tensor("qt2", [P, SL], BF16, kind="ExternalInput")
    kt2 = nc.dram_tensor("kt2", [P, SL], BF16, kind="ExternalInput")
    v1 = nc.dram_tensor("v1", [P, NKB * (D + 1)], BF16, kind="ExternalInput")
    dmask = nc.dram_tensor("dmask", [P, 2 * P], BF16, kind="ExternalInput")
    if with_padding:
        biasm = nc.dram_tensor("biasm", [P, NKB], F32, kind="ExternalInput")
    out = nc.dram_tensor("out", [SL, D], F32, kind="ExternalOutput")

    with tile.TileContext(nc) as tc:
        with (
            tc.tile_pool(name="const", bufs=1) as constp,
            tc.tile_pool(name="spool", bufs=2, space="PSUM") as spool,
            tc.tile_pool(name="opsum", bufs=1, space="PSUM") as opsum,
            tc.tile_pool(name="epool", bufs=8) as epool,
            tc.tile_pool(name="npool", bufs=4) as npool,
        ):
            # Chunked input loads on separate DGE queues so the first S^T
            # matmul is gated only on the first kt/qt chunks, not the full
            # tensors. kt chunks on SP, qt chunks on ACT, mask+V on GPSIMD.
            qt_t = constp.tile([P, SL], BF16, tag="qt")
            kt_t = constp.tile([P, SL], BF16, tag="kt")
            for c in range(0, SL, 512):
                nc.sync.dma_start(kt_t[:, c:c + 512], kt2[:, c:c + 512])
                nc.scalar.dma_start(qt_t[:, c:c + 512], qt2[:, c:c + 512])
            dm_t = constp.tile([P, 2 * P], BF16, tag="dmask")
            nc.gpsimd.dma_start(dm_t[:], dmask[:])
            v1_t = constp.tile([P, NKB, D + 1], BF16, tag="v1")
            nc.gpsimd.dma_start(
                v1_t[:], v1[:].rearrange("p (kb c) -> p kb c", kb=NKB)
            )
            if with_padding:
                bm_t = constp.tile([P, NKB], F32, tag="biasm")
                nc.scalar.dma_start(bm_t[:], biasm[:])

            # 4 PSUM banks, 4 query-block accumulators [128, 65] each at
            # col offsets 0/65/130/195. One accumulation group per bank
            # (PSUM zero regions are bank-granular): start on the bank's
            # first matmul, stop on its last. Uniform 4-per-bank measured
            # ~3us faster than tail-friendly uneven packings.
            BANK_OF = [qb // 4 for qb in range(NQB)]
            BANK_START = [0, 4, 8, 12]
            BANK_END = [3, 7, 11, 15]
            ob = [
                opsum.tile([P, 512], F32, tag=f"ob{j}", name=f"ob{j}")
                for j in range(4)
            ]

            def emit_pv(j):
                # PV matmuls for key pair j: qb = j..15, even then odd kb.
                q0 = j * P
                extent = SL - q0
                ch = _chunks(extent)
                for parity in range(2):
                    kb = 2 * j + parity
                    for qb in range(j, NQB):
                        off = (qb - j) * P
                        # locate chunk containing off
                        for ci, (c0, clen) in enumerate(ch):
                            if c0 <= off < c0 + clen:
                                break
                        e = e_tiles[(j % 2, ci)]
                        col = parity * 512 + (off - c0)
                        bank = BANK_OF[qb]
                        slot = qb - BANK_START[bank]
                        nc.tensor.matmul(
                            ob[bank][:, slot * 65: slot * 65 + 65],
                            e[:, col: col + P],
                            v1_t[:, kb, :],
                            start=(j == 0 and parity == 0 and slot == 0),
                            stop=(j == BANK_END[bank] and parity == 1
                                  and qb == BANK_END[bank]),
                        )

            def emit_norm(qb):
                bank = BANK_OF[qb]
                slot = qb - BANK_START[bank]
                r = npool.tile([P, 1], F32, tag="r", name="r")
                nc.vector.reciprocal(
                    r[:], ob[bank][:, slot * 65 + 64: slot * 65 + 65]
                )
                o = npool.tile([P, D], F32, tag="o", name="o")
                nc.vector.tensor_scalar_mul(
                    o[:], ob[bank][:, slot * 65: slot * 65 + 64], r[:]
                )
                nc.sync.dma_start(out[qb * P:(qb + 1) * P, :], o[:])

            # main software-pipelined loop over key-block pairs
            e_tiles = {}
            pending_norms = []
            for j in range(NPAIR):
                q0 = j * P
                extent = SL - q0
                ch = _chunks(extent)
                # S^T for pair j: two row-group-packed matmuls per chunk
                # (even kb on PE rows 0-63, odd kb on rows 64-127)
                ps_tiles = []
                for ci, (c0, clen) in enumerate(ch):
                    ps = spool.tile([P, 1024], F32, tag="ps", name="ps")
                    ps_tiles.append(ps)
                    for parity in range(2):
                        lo = parity * 64
                        nc.tensor.matmul(
                            ps[:, parity * 512: parity * 512 + clen],
                            kt_t[lo:lo + 64, j * P:(j + 1) * P],
                            qt_t[lo:lo + 64, q0 + c0: q0 + c0 + clen],
                            start=True, stop=True,
                        )
                # PV for the previous pair (keeps PE busy during ACT(j)).
                if j > 0:
                    emit_pv(j - 1)
                    for bank in range(4):
                        if BANK_END[bank] == j - 1:
                            pending_norms.extend(
                                range(BANK_START[bank], BANK_END[bank] + 1))
                # Normalizations are spread out (max 2 per pair) so the DVE
                # queue never delays the diag-mask op by a long norm burst.
                for _ in range(2):
                    if pending_norms:
                        emit_norm(pending_norms.pop(0))
                # exp for pair j
                for ci, (c0, clen) in enumerate(ch):
                    ps = ps_tiles[ci]
                    e = epool.tile([P, 1024], BF16, tag="e", name="e")
                    e_tiles[(j % 2, ci)] = e
                    if with_padding:
                        # separate exp per kb: bias differs per parity
                        for parity in range(2):
                            nc.scalar.activation(
                                e[:, parity * 512: parity * 512 + clen],
                                ps[:, parity * 512: parity * 512 + clen],
                                mybir.ActivationFunctionType.Exp,
                                bias=bm_t[:, 2 * j + parity: 2 * j + parity + 1],
                                scale=float(SCALE),
                            )
                    else:
                        if clen == 512:
                            src, dst = ps[:, :1024], e[:, :1024]
                        else:
                            # strided AP: [0:clen] and [512:512+clen]
                            src = ps[:].rearrange(
                                "p (two f) -> p two f", two=2)[:, :, :clen]
                            dst = e[:].rearrange(
                                "p (two f) -> p two f", two=2)[:, :, :clen]
                        nc.scalar.activation(
                            dst, src,
                            mybir.ActivationFunctionType.Exp,
                            bias=0.0,
                            scale=float(SCALE),
                        )
                # causal mask on the two diagonal blocks (first 128 q cols):
                # chunk 0 holds them at cols [0:128] (even) / [512:640] (odd)
                e0 = e_tiles[(j % 2, 0)]
                ea = e0[:].rearrange("p (two f) -> p two f", two=2)[:, :, :P]
                ma = dm_t[:].rearrange("p (two f) -> p two f", two=2)
                nc.vector.tensor_mul(ea, ea, ma)

            emit_pv(NPAIR - 1)
            for bank in range(4):
                if BANK_END[bank] == NPAIR - 1:
                    pending_norms.extend(
                        range(BANK_START[bank], BANK_END[bank] + 1))
            for qb in pending_norms:
                emit_norm(qb)

    nc.compile()
    return nc


def _get_program(with_padding: bool):
    key = ("prog", with_padding)
    if key not in _cache:
        _cache[key] = _build_program(with_padding)
    return _cache[key]


def _diag_masks():
    # dmask[:, 0:128]  : key block 2*qb   -> visible iff u <= 2r+p
    # dmask[:, 128:256]: key block 2*qb+1 -> visible iff u+128 <= 2r+p
    u = np.arange(P)[:, None]
    r = np.arange(P)[None, :]
    out = []
    for p in range(2):
        m0 = (u <= 2 * r + p)
        m1 = (u + P <= 2 * r + p)
        out.append(np.concatenate([m0, m1], axis=1).astype(NP_BF16))
    return out


def kernel(query, key, value, attention_mask, _run_opts=None):
    query = np.asarray(query, dtype=np.float32)
    key = np.asarray(key, dtype=np.float32)
    value = np.asarray(value, dtype=np.float32)
    attention_mask = np.asarray(attention_mask)

    with_padding = not bool((attention_mask != 0).all())
    nc = _get_program(with_padding)
    masks = _diag_masks()

    in_maps = []
    for b in range(B):
        kt = key[b].T  # [64, 4096]
        # kt2: top half = even key blocks, bottom half = odd key blocks
        ktb = kt.reshape(D, NPAIR, 2, P)
        kt2_b = np.ascontiguousarray(
            np.concatenate([ktb[:, :, 0, :], ktb[:, :, 1, :]], axis=0)
            .reshape(2 * D, SL).astype(NP_BF16))
        # v1: [128, 32*65], row p = concat over kb of [V[kb*128+p, :], 1]
        v1f = np.concatenate(
            [value[b].reshape(NKB, P, D),
             np.ones((NKB, P, 1), np.float32)], axis=2)      # [32, 128, 65]
        v1_b = np.ascontiguousarray(
            v1f.transpose(1, 0, 2).reshape(P, NKB * (D + 1)).astype(NP_BF16))
        if with_padding:
            bias_b = np.where(
                attention_mask[b] != 0, 0.0, PAD_BIAS).astype(np.float32)
            biasm_b = np.ascontiguousarray(bias_b.reshape(NKB, P).T)
        for p in range(2):
            qt = query[b, p::2].T.astype(NP_BF16)            # [64, 2048]
            qt2_b = np.ascontiguousarray(np.concatenate([qt, qt], axis=0))
            m = {
                "qt2": qt2_b,
                "kt2": kt2_b,
                "v1": v1_b,
                "dmask": masks[p],
            }
            if with_padding:
                m["biasm"] = biasm_b
            in_maps.append(m)

    run_opts = _run_opts or {}
    res = run_bass_kernel_spmd(nc, in_maps, core_ids=list(range(N_CORES)),
                               **run_opts)
    if run_opts:
        _cache["last_results"] = res

    out = np.empty((B, S, D), np.float32)
    for i in range(N_CORES):
        b, p = divmod(i, 2)
        out[b, p::2] = res.results[i]["out"]
    return out


# revision 19
# speedup vs baseline: 1.0909x; 1.0245x over previous
"""Causal attention (B=4, S=4096, D=64, fp32) on 8 TRN2 NeuronCores.

Sharding: 8 cores = 4 batches x 2 query-parity shards. Core (b, p) handles
query rows  q_global = 2*i + p  of batch b (i = 0..2047). This interleaved
split makes the causal extent pattern identical on every core (SPMD-uniform):
local query block qb (128 rows) attends exactly key blocks 0..2*qb+1, so key
blocks are processed in PAIRS j = (2j, 2j+1), both with extent [128j, 2048).

Per-core kernel (matmul operands bf16, accumulation fp32), loop over j=0..15:
  S^T[k, q] for kb=2j / 2j+1 as two row-group-packed matmuls (K=64 each,
    PE rows 0-63 / 64-127 concurrently), outputs side by side in one PSUM
    tile [128, A|B]                                              (PE)
  E = exp(S^T * 0.125 [+ pad bias])   one ACT op per chunk covers both kb
  E[:, diag] *= causal mask           (DVE, strided AP hits both kb)
  PV[q, 0:65] += E_even^T @ [V|1] + E_odd^T @ [V|1]  (PE; ones col = Z)
  out[q, :] = PV[q, :64] * (1 / PV[q, 64])

Emission is software-pipelined: S^T(j) is issued before PV(j-1) so the PE
queue always has independent work while ACT(j-1) finishes.

No max-subtraction: scaled scores are ~N(0,1), exp is safe in fp32. The
softmax denominator comes from the ones column, so numerator and denominator
use identical bf16 weights.
"""

import numpy as np
import ml_dtypes

import concourse.bass as bass
import concourse.bacc as bacc
import concourse.mybir as mybir
import concourse.tile as tile
from concourse.bass_utils import run_bass_kernel_spmd

BF16 = mybir.dt.bfloat16
F32 = mybir.dt.float32
NP_BF16 = ml_dtypes.bfloat16

B, S, D = 4, 4096, 64
P = 128
SL = S // 2          # local query count per core
NKB = S // P         # 32 key blocks
NPAIR = NKB // 2     # 16 key-block pairs
NQB = SL // P        # 16 local query blocks
SCALE = 1.0 / np.sqrt(D)
PAD_BIAS = -50.0     # additive pre-exp bias for padded-out keys
N_CORES = 8

_cache: dict = {}


def _chunks(extent):
    """Split [0, extent) into a leading remainder chunk (if any) plus full
    512-col chunks, so every chunk boundary is 512-aligned from the top."""
    rem = extent % 512
    out = []
    c = 0
    if rem:
        out.append((0, rem))
        c = rem
    while c < extent:
        out.append((c, 512))
        c += 512
    return out


def _build_program(with_padding: bool):
    nc = bacc.Bacc("TRN2", debug=False)

    # Host pre-layouts (see kernel()):
    #  qt2 [128, 2048]: rows 0-63 = Q^T, rows 64-127 = the same Q^T again
    #  kt2 [128, 2048]: rows 0-63 = K^T of even key blocks, 64-127 = odd
    #  v1  [128, 32*65]: row p = concat_kb [V[kb*128+p, :], 1.0]
    qt2 = nc.dram_tensor("qt2", [P, SL], BF16, kind="ExternalInput")
    kt2 = nc.dram_tensor("kt2", [P, SL], BF16, kind="ExternalInput")
    v1 = nc.dram_tensor("v1", [P, NKB * (D + 1)], BF16, kind="ExternalInput")
    dmask = nc.dram_tensor("dmask", [P, 2 * P], BF16, kind="ExternalInput")
    if with_padding:
        biasm = nc.dram_tensor("biasm", [P, NKB], F32, kind="ExternalInput")
    # unnormalized output + Z column, row-major by local partition r:
    # out[r, qb*65+c] = sum_k E[k, qb*128+r] * V1[k, c]  (host divides)
    out = nc.dram_tensor("out", [P, NQB * (D + 1)], BF16,
                         kind="ExternalOutput")

    with tile.TileContext(nc) as tc:
        with (
            tc.tile_pool(name="const", bufs=1) as constp,
            tc.tile_pool(name="spool", bufs=2, space="PSUM") as spool,
            tc.tile_pool(name="opsum", bufs=1, space="PSUM") as opsum,
            tc.tile_pool(name="epool", bufs=8) as epool,
            tc.tile_pool(name="npool", bufs=4) as npool,
        ):
            # Chunked input loads on separate DGE queues so the first S^T
            # matmul is gated only on the first kt/qt chunks, not the full
            # tensors. kt chunks on SP, qt chunks on ACT, mask+V on GPSIMD.
            qt_t = constp.tile([P, SL], BF16, tag="qt")
            kt_t = constp.tile([P, SL], BF16, tag="kt")
            # need order: kt[0:256] + qt chunks (pair 0 spans all of qt),
            # then the rest of kt (pair j needs kt cols 128j:128j+128)
            nc.scalar.dma_start(kt_t[:, 0:256], kt2[:, 0:256])
            nc.sync.dma_start(qt_t[:, 0:512], qt2[:, 0:512])
            nc.scalar.dma_start(qt_t[:, 512:1024], qt2[:, 512:1024])
            nc.sync.dma_start(qt_t[:, 1024:1536], qt2[:, 1024:1536])
            nc.scalar.dma_start(qt_t[:, 1536:2048], qt2[:, 1536:2048])
            nc.sync.dma_start(kt_t[:, 256:1024], kt2[:, 256:1024])
            nc.scalar.dma_start(kt_t[:, 1024:2048], kt2[:, 1024:2048])
            dm_t = constp.tile([P, 2 * P], BF16, tag="dmask")
            nc.gpsimd.dma_start(dm_t[:], dmask[:])
            v1_t = constp.tile([P, NKB, D + 1], BF16, tag="v1")
            nc.gpsimd.dma_start(
                v1_t[:], v1[:].rearrange("p (kb c) -> p kb c", kb=NKB)
            )
            if with_padding:
                bm_t = constp.tile([P, NKB], F32, tag="biasm")
                nc.scalar.dma_start(bm_t[:], biasm[:])

            # 4 PSUM banks, 4 query-block accumulators [128, 65] each at
            # col offsets 0/65/130/195. One accumulation group per bank
            # (PSUM zero regions are bank-granular): start on the bank's
            # first matmul, stop on its last. Uniform 4-per-bank measured
            # ~3us faster than tail-friendly uneven packings.
            BANK_OF = [qb // 4 for qb in range(NQB)]
            BANK_START = [0, 4, 8, 12]
            BANK_END = [3, 7, 11, 15]
            ob = [
                opsum.tile([P, 512], F32, tag=f"ob{j}", name=f"ob{j}")
                for j in range(4)
            ]

            def emit_pv(j):
                # PV matmuls for key pair j: qb = j..15, even then odd kb.
                q0 = j * P
                extent = SL - q0
                ch = _chunks(extent)
                for parity in range(2):
                    kb = 2 * j + parity
                    for qb in range(j, NQB):
                        off = (qb - j) * P
                        # locate chunk containing off
                        for ci, (c0, clen) in enumerate(ch):
                            if c0 <= off < c0 + clen:
                                break
                        e = e_tiles[(j % 2, ci)]
                        col = parity * 512 + (off - c0)
                        bank = BANK_OF[qb]
                        slot = qb - BANK_START[bank]
                        nc.tensor.matmul(
                            ob[bank][:, slot * 65: slot * 65 + 65],
                            e[:, col: col + P],
                            v1_t[:, kb, :],
                            start=(j == 0 and parity == 0 and slot == 0),
                            stop=(j == BANK_END[bank] and parity == 1
                                  and qb == BANK_END[bank]),
                        )

            def emit_store(bank):
                # bank's accumulation group is closed: copy all 4
                # accumulators [128, 260] to SBUF bf16 in one DVE op and
                # store with 520B-per-partition-row DMA packets. The
                # softmax division happens on the host.
                w = 4 * 65
                o = npool.tile([P, w], BF16, tag="o", name="o")
                nc.vector.tensor_copy(o[:], ob[bank][:, :w])
                nc.sync.dma_start(
                    out[:, bank * w:(bank + 1) * w], o[:])

            # main software-pipelined loop over key-block pairs
            e_tiles = {}
            for j in range(NPAIR):
                q0 = j * P
                extent = SL - q0
                ch = _chunks(extent)
                # S^T for pair j: two row-group-packed matmuls per chunk
                # (even kb on PE rows 0-63, odd kb on rows 64-127)
                ps_tiles = []
                for ci, (c0, clen) in enumerate(ch):
                    ps = spool.tile([P, 1024], F32, tag="ps", name="ps")
                    ps_tiles.append(ps)
                    for parity in range(2):
                        lo = parity * 64
                        nc.tensor.matmul(
                            ps[:, parity * 512: parity * 512 + clen],
                            kt_t[lo:lo + 64, j * P:(j + 1) * P],
                            qt_t[lo:lo + 64, q0 + c0: q0 + c0 + clen],
                            start=True, stop=True,
                        )
                # PV for the previous pair (keeps PE busy during ACT(j)).
                if j > 0:
                    emit_pv(j - 1)
                    for bank in range(4):
                        if BANK_END[bank] == j - 1:
                            emit_store(bank)
                # exp for pair j
                for ci, (c0, clen) in enumerate(ch):
                    ps = ps_tiles[ci]
                    e = epool.tile([P, 1024], BF16, tag="e", name="e")
                    e_tiles[(j % 2, ci)] = e
                    if with_padding:
                        # separate exp per kb: bias differs per parity
                        for parity in range(2):
                            nc.scalar.activation(
                                e[:, parity * 512: parity * 512 + clen],
                                ps[:, parity * 512: parity * 512 + clen],
                                mybir.ActivationFunctionType.Exp,
                                bias=bm_t[:, 2 * j + parity: 2 * j + parity + 1],
                                scale=float(SCALE),
                            )
                    else:
                        if clen == 512:
                            src, dst = ps[:, :1024], e[:, :1024]
                        else:
                            # strided AP: [0:clen] and [512:512+clen]
                            src = ps[:].rearrange(
                                "p (two f) -> p two f", two=2)[:, :, :clen]
                            dst = e[:].rearrange(
                                "p (two f) -> p two f", two=2)[:, :, :clen]
                        nc.scalar.activation(
                            dst, src,
                            mybir.ActivationFunctionType.Exp,
                            bias=0.0,
                            scale=float(SCALE),
                        )
                # causal mask on the two diagonal blocks (first 128 q cols):
                # chunk 0 holds them at cols [0:128] (even) / [512:640] (odd)
                e0 = e_tiles[(j % 2, 0)]
                ea = e0[:].rearrange("p (two f) -> p two f", two=2)[:, :, :P]
                ma = dm_t[:].rearrange("p (two f) -> p two f", two=2)
                nc.vector.tensor_mul(ea, ea, ma)

            emit_pv(NPAIR - 1)
            emit_store(3)

    nc.compile()
    return nc


def _get_program(with_padding: bool):
    key = ("prog", with_padding)
    if key not in _cache:
        _cache[key] = _build_program(with_padding)
    return _cache[key]


def _diag_masks():
    # dmask[:, 0:128]  : key block 2*qb   -> visible iff u <= 2r+p
    # dmask[:, 128:256]: key block 2*qb+1 -> visible iff u+128 <= 2r+p
    u = np.arange(P)[:, None]
    r = np.arange(P)[None, :]
    out = []
    for p in range(2):
        m0 = (u <= 2 * r + p)
        m1 = (u + P <= 2 * r + p)
        out.append(np.concatenate([m0, m1], axis=1).astype(NP_BF16))
    return out


def kernel(query, key, value, attention_mask, _run_opts=None):
    query = np.asarray(query, dtype=np.float32)
    key = np.asarray(key, dtype=np.float32)
    value = np.asarray(value, dtype=np.float32)
    attention_mask = np.asarray(attention_mask)

    with_padding = not bool((attention_mask != 0).all())
    nc = _get_program(with_padding)
    masks = _diag_masks()

    in_maps = []
    for b in range(B):
        kt = key[b].T  # [64, 4096]
        # kt2: top half = even key blocks, bottom half = odd key blocks
        ktb = kt.reshape(D, NPAIR, 2, P)
        kt2_b = np.ascontiguousarray(
            np.concatenate([ktb[:, :, 0, :], ktb[:, :, 1, :]], axis=0)
            .reshape(2 * D, SL).astype(NP_BF16))
        # v1: [128, 32*65], row p = concat over kb of [V[kb*128+p, :], 1]
        v1f = np.concatenate(
            [value[b].reshape(NKB, P, D),
             np.ones((NKB, P, 1), np.float32)], axis=2)      # [32, 128, 65]
        v1_b = np.ascontiguousarray(
            v1f.transpose(1, 0, 2).reshape(P, NKB * (D + 1)).astype(NP_BF16))
        if with_padding:
            bias_b = np.where(
                attention_mask[b] != 0, 0.0, PAD_BIAS).astype(np.float32)
            biasm_b = np.ascontiguousarray(bias_b.reshape(NKB, P).T)
        for p in range(2):
            qt = query[b, p::2].T.astype(NP_BF16)            # [64, 2048]
            qt2_b = np.ascontiguousarray(np.concatenate([qt, qt], axis=0))
            m = {
                "qt2": qt2_b,
                "kt2": kt2_b,
                "v1": v1_b,
                "dmask": masks[p],
            }
            if with_padding:
                m["biasm"] = biasm_b
            in_maps.append(m)

    run_opts = _run_opts or {}
    res = run_bass_kernel_spmd(nc, in_maps, core_ids=list(range(N_CORES)),
                               **run_opts)
    if run_opts:
        _cache["last_results"] = res

    out = np.empty((B, S, D), np.float32)
    for i in range(N_CORES):
        b, p = divmod(i, 2)
        buf = res.results[i]["out"].astype(np.float32)  # [128, 16*65]
        buf = buf.reshape(P, NQB, D + 1).transpose(1, 0, 2).reshape(SL, D + 1)
        out[b, p::2] = buf[:, :D] / buf[:, D:D + 1]
    return out


# revision 20
# speedup vs baseline: 1.1153x; 1.0224x over previous
"""Causal attention (B=4, S=4096, D=64, fp32) on 8 TRN2 NeuronCores.

Sharding: 8 cores = 4 batches x 2 query-parity shards. Core (b, p) handles
query rows  q_global = 2*i + p  of batch b (i = 0..2047). This interleaved
split makes the causal extent pattern identical on every core (SPMD-uniform):
local query block qb (128 rows) attends exactly key blocks 0..2*qb+1, so key
blocks are processed in PAIRS j = (2j, 2j+1), both with extent [128j, 2048).

Per-core kernel (matmul operands bf16, accumulation fp32), loop over j=0..15:
  S^T[k, q] for kb=2j / 2j+1 as two row-group-packed matmuls (K=64 each,
    PE rows 0-63 / 64-127 concurrently), outputs side by side in one PSUM
    tile [128, A|B]                                              (PE)
  E = exp(S^T * 0.125 [+ pad bias])   one ACT op per chunk covers both kb
  E[:, diag] *= causal mask           (DVE, strided AP hits both kb)
  PV[q, 0:65] += E_even^T @ [V|1] + E_odd^T @ [V|1]  (PE; ones col = Z)
  out[q, :] = PV[q, :64] * (1 / PV[q, 64])

Emission is software-pipelined: S^T(j) is issued before PV(j-1) so the PE
queue always has independent work while ACT(j-1) finishes.

No max-subtraction: scaled scores are ~N(0,1), exp is safe in fp32. The
softmax denominator comes from the ones column, so numerator and denominator
use identical bf16 weights.
"""

import numpy as np
import ml_dtypes

import concourse.bass as bass
import concourse.bacc as bacc
import concourse.mybir as mybir
import concourse.tile as tile
from concourse.bass_utils import run_bass_kernel_spmd

BF16 = mybir.dt.bfloat16
F32 = mybir.dt.float32
NP_BF16 = ml_dtypes.bfloat16

B, S, D = 4, 4096, 64
P = 128
SL = S // 2          # local query count per core
NKB = S // P         # 32 key blocks
NPAIR = NKB // 2     # 16 key-block pairs
NQB = SL // P        # 16 local query blocks
SCALE = 1.0 / np.sqrt(D)
PAD_BIAS = -50.0     # additive pre-exp bias for padded-out keys
N_CORES = 8

_cache: dict = {}


def _chunks(extent):
    """Split [0, extent) into a leading remainder chunk (if any) plus full
    512-col chunks, so every chunk boundary is 512-aligned from the top."""
    rem = extent % 512
    out = []
    c = 0
    if rem:
        out.append((0, rem))
        c = rem
    while c < extent:
        out.append((c, 512))
        c += 512
    return out


def _build_program(with_padding: bool):
    nc = bacc.Bacc("TRN2", debug=False)

    # Host pre-layouts (see kernel()):
    #  qt2 [128, 2048]: rows 0-63 = Q^T, rows 64-127 = the same Q^T again
    #  kt2 [128, 2048]: rows 0-63 = K^T of even key blocks, 64-127 = odd
    #  v1  [128, 32*65]: row p = concat_kb [V[kb*128+p, :], 1.0]
    qt2 = nc.dram_tensor("qt2", [P, SL], BF16, kind="ExternalInput")
    kt2 = nc.dram_tensor("kt2", [P, SL], BF16, kind="ExternalInput")
    v1 = nc.dram_tensor("v1", [P, NKB * (D + 1)], BF16, kind="ExternalInput")
    dmask = nc.dram_tensor("dmask", [P, 2 * P], BF16, kind="ExternalInput")
    if with_padding:
        biasm = nc.dram_tensor("biasm", [P, NKB], F32, kind="ExternalInput")
    # unnormalized output + Z column, row-major by local partition r:
    # out[r, qb*65+c] = sum_k E[k, qb*128+r] * V1[k, c]  (host divides)
    out = nc.dram_tensor("out", [P, NQB * (D + 1)], BF16,
                         kind="ExternalOutput")

    with tile.TileContext(nc) as tc:
        with (
            tc.tile_pool(name="const", bufs=1) as constp,
            tc.tile_pool(name="spool", bufs=2, space="PSUM") as spool,
            tc.tile_pool(name="opsum", bufs=1, space="PSUM") as opsum,
            tc.tile_pool(name="epool", bufs=8) as epool,
            tc.tile_pool(name="npool", bufs=4) as npool,
        ):
            # Chunked input loads on separate DGE queues so the first S^T
            # matmul is gated only on the first kt/qt chunks, not the full
            # tensors. kt chunks on SP, qt chunks on ACT, mask+V on GPSIMD.
            qt_t = constp.tile([P, SL], BF16, tag="qt")
            kt_t = constp.tile([P, SL], BF16, tag="kt")
            # Two-chunk loads: per-packet DMA cost is high, so keep 2KB
            # packets while still letting pair 0 start on the first chunks.
            nc.scalar.dma_start(kt_t[:, 0:1024], kt2[:, 0:1024])
            nc.sync.dma_start(qt_t[:, 0:1024], qt2[:, 0:1024])
            nc.sync.dma_start(qt_t[:, 1024:2048], qt2[:, 1024:2048])
            nc.scalar.dma_start(kt_t[:, 1024:2048], kt2[:, 1024:2048])
            dm_t = constp.tile([P, 2 * P], BF16, tag="dmask")
            nc.gpsimd.dma_start(dm_t[:], dmask[:])
            v1_t = constp.tile([P, NKB, D + 1], BF16, tag="v1")
            half = NKB // 2
            for h in range(2):
                nc.gpsimd.dma_start(
                    v1_t[:, h * half:(h + 1) * half, :],
                    v1[:, h * half * (D + 1):(h + 1) * half * (D + 1)]
                    .rearrange("p (kb c) -> p kb c", kb=half),
                )
            if with_padding:
                bm_t = constp.tile([P, NKB], F32, tag="biasm")
                nc.scalar.dma_start(bm_t[:], biasm[:])

            # 4 PSUM banks, 4 query-block accumulators [128, 65] each at
            # col offsets 0/65/130/195. One accumulation group per bank
            # (PSUM zero regions are bank-granular): start on the bank's
            # first matmul, stop on its last. Uniform 4-per-bank measured
            # ~3us faster than tail-friendly uneven packings.
            BANK_OF = [qb // 4 for qb in range(NQB)]
            BANK_START = [0, 4, 8, 12]
            BANK_END = [3, 7, 11, 15]
            ob = [
                opsum.tile([P, 512], F32, tag=f"ob{j}", name=f"ob{j}")
                for j in range(4)
            ]

            def emit_pv(j):
                # PV matmuls for key pair j: qb = j..15, even then odd kb.
                q0 = j * P
                extent = SL - q0
                ch = _chunks(extent)
                for parity in range(2):
                    kb = 2 * j + parity
                    for qb in range(j, NQB):
                        off = (qb - j) * P
                        # locate chunk containing off
                        for ci, (c0, clen) in enumerate(ch):
                            if c0 <= off < c0 + clen:
                                break
                        e = e_tiles[(j % 2, ci)]
                        col = parity * 512 + (off - c0)
                        bank = BANK_OF[qb]
                        slot = qb - BANK_START[bank]
                        nc.tensor.matmul(
                            ob[bank][:, slot * 65: slot * 65 + 65],
                            e[:, col: col + P],
                            v1_t[:, kb, :],
                            start=(j == 0 and parity == 0 and slot == 0),
                            stop=(j == BANK_END[bank] and parity == 1
                                  and qb == BANK_END[bank]),
                        )

            def emit_store(bank):
                # bank's accumulation group is closed: copy all 4
                # accumulators [128, 260] to SBUF bf16 in one DVE op and
                # store with 520B-per-partition-row DMA packets. The
                # softmax division happens on the host.
                w = 4 * 65
                o = npool.tile([P, w], BF16, tag="o", name="o")
                nc.vector.tensor_copy(o[:], ob[bank][:, :w])
                nc.sync.dma_start(
                    out[:, bank * w:(bank + 1) * w], o[:])

            # main software-pipelined loop over key-block pairs
            e_tiles = {}
            for j in range(NPAIR):
                q0 = j * P
                extent = SL - q0
                ch = _chunks(extent)
                # S^T for pair j: two row-group-packed matmuls per chunk
                # (even kb on PE rows 0-63, odd kb on rows 64-127)
                ps_tiles = []
                for ci, (c0, clen) in enumerate(ch):
                    ps = spool.tile([P, 1024], F32, tag="ps", name="ps")
                    ps_tiles.append(ps)
                    for parity in range(2):
                        lo = parity * 64
                        nc.tensor.matmul(
                            ps[:, parity * 512: parity * 512 + clen],
                            kt_t[lo:lo + 64, j * P:(j + 1) * P],
                            qt_t[lo:lo + 64, q0 + c0: q0 + c0 + clen],
                            start=True, stop=True,
                        )
                # PV for the previous pair (keeps PE busy during ACT(j)).
                if j > 0:
                    emit_pv(j - 1)
                    for bank in range(4):
                        if BANK_END[bank] == j - 1:
                            emit_store(bank)
                # exp for pair j
                for ci, (c0, clen) in enumerate(ch):
                    ps = ps_tiles[ci]
                    e = epool.tile([P, 1024], BF16, tag="e", name="e")
                    e_tiles[(j % 2, ci)] = e
                    if with_padding:
                        # separate exp per kb: bias differs per parity
                        for parity in range(2):
                            nc.scalar.activation(
                                e[:, parity * 512: parity * 512 + clen],
                                ps[:, parity * 512: parity * 512 + clen],
                                mybir.ActivationFunctionType.Exp,
                                bias=bm_t[:, 2 * j + parity: 2 * j + parity + 1],
                                scale=float(SCALE),
                            )
                    else:
                        if clen == 512:
                            src, dst = ps[:, :1024], e[:, :1024]
                        else:
                            # strided AP: [0:clen] and [512:512+clen]
                            src = ps[:].rearrange(
                                "p (two f) -> p two f", two=2)[:, :, :clen]
                            dst = e[:].rearrange(
                                "p (two f) -> p two f", two=2)[:, :, :clen]
                        nc.scalar.activation(
                            dst, src,
                            mybir.ActivationFunctionType.Exp,
                            bias=0.0,
                            scale=float(SCALE),
                        )
                # causal mask on the two diagonal blocks (first 128 q cols):
                # chunk 0 holds them at cols [0:128] (even) / [512:640] (odd)
                e0 = e_tiles[(j % 2, 0)]
                ea = e0[:].rearrange("p (two f) -> p two f", two=2)[:, :, :P]
                ma = dm_t[:].rearrange("p (two f) -> p two f", two=2)
                nc.vector.tensor_mul(ea, ea, ma)

            emit_pv(NPAIR - 1)
            emit_store(3)

    nc.compile()
    return nc


def _get_program(with_padding: bool):
    key = ("prog", with_padding)
    if key not in _cache:
        _cache[key] = _build_program(with_padding)
    return _cache[key]


def _diag_masks():
    # dmask[:, 0:128]  : key block 2*qb   -> visible iff u <= 2r+p
    # dmask[:, 128:256]: key block 2*qb+1 -> visible iff u+128 <= 2r+p
    u = np.arange(P)[:, None]
    r = np.arange(P)[None, :]
    out = []
    for p in range(2):
        m0 = (u <= 2 * r + p)
        m1 = (u + P <= 2 * r + p)
        out.append(np.concatenate([m0, m1], axis=1).astype(NP_BF16))
    return out


def kernel(query, key, value, attention_mask, _run_opts=None):
    query = np.asarray(query, dtype=np.float32)
    key = np.asarray(key, dtype=np.float32)
    value = np.asarray(value, dtype=np.float32)
    attention_mask = np.asarray(attention_mask)

    with_padding = not bool((attention_mask != 0).all())
    nc = _get_program(with_padding)
    masks = _diag_masks()

    in_maps = []
    for b in range(B):
        kt = key[b].T  # [64, 4096]
        # kt2: top half = even key blocks, bottom half = odd key blocks
        ktb = kt.reshape(D, NPAIR, 2, P)
        kt2_b = np.ascontiguousarray(
            np.concatenate([ktb[:, :, 0, :], ktb[:, :, 1, :]], axis=0)
            .reshape(2 * D, SL).astype(NP_BF16))
        # v1: [128, 32*65], row p = concat over kb of [V[kb*128+p, :], 1]
        v1f = np.concatenate(
            [value[b].reshape(NKB, P, D),
             np.ones((NKB, P, 1), np.float32)], axis=2)      # [32, 128, 65]
        v1_b = np.ascontiguousarray(
            v1f.transpose(1, 0, 2).reshape(P, NKB * (D + 1)).astype(NP_BF16))
        if with_padding:
            bias_b = np.where(
                attention_mask[b] != 0, 0.0, PAD_BIAS).astype(np.float32)
            biasm_b = np.ascontiguousarray(bias_b.reshape(NKB, P).T)
        for p in range(2):
            qt = query[b, p::2].T.astype(NP_BF16)            # [64, 2048]
            qt2_b = np.ascontiguousarray(np.concatenate([qt, qt], axis=0))
            m = {
                "qt2": qt2_b,
                "kt2": kt2_b,
                "v1": v1_b,
                "dmask": masks[p],
            }
            if with_padding:
                m["biasm"] = biasm_b
            in_maps.append(m)

    run_opts = _run_opts or {}
    res = run_bass_kernel_spmd(nc, in_maps, core_ids=list(range(N_CORES)),
                               **run_opts)
    if run_opts:
        _cache["last_results"] = res

    out = np.empty((B, S, D), np.float32)
    for i in range(N_CORES):
        b, p = divmod(i, 2)
        buf = res.results[i]["out"].astype(np.float32)  # [128, 16*65]
        buf = buf.reshape(P, NQB, D + 1).transpose(1, 0, 2).reshape(SL, D + 1)
        out[b, p::2] = buf[:, :D] / buf[:, D:D + 1]
    return out
